# revision 1
# baseline (speedup 1.0000x reference)
"""GQA (RoPE + causal softmax) Trainium2 Bass kernel, 8-core SPMD.

Sharding: DP over batch (2) x TP over KV groups (4 quarters of heads).
Core c handles batch c//4 and head quarter c%4 (8 q-heads, 2 kv-heads).
Each core computes a partial o_proj ([S, D]); host sums 4 partials per batch.

All matmuls run in float32r (TF32-like, 1 cyc/row at N>=256).
Everything on-chip is kept in "transposed" layout (feature dim on
partitions), which makes x^T the only host-side layout prep needed.
"""

import numpy as np

import concourse.bass as bass
import concourse.mybir as mybir
import concourse.tile as tile
from concourse import bacc, bass_utils

B, S, D = 2, 2048, 2048
H, KV, HD = 32, 8, 64
REP = H // KV
SCALE = 1.0 / 8.0  # 1/sqrt(HD)

F32 = mybir.dt.float32
F32R = mybir.dt.float32r
EXP = mybir.ActivationFunctionType.Exp

NCHUNK = S // 512        # 4 sq chunks of 512
NKT = D // 128           # 16 k-tiles over D
NST = S // 128           # 16 sk/s tiles

# local head j (0..7) -> denom row
def _pairrow(j):
    return 2 * (j % 4) + (j // 4)


def _build_program():
    nc = bacc.Bacc()

    xT = nc.dram_tensor("xT", [D, S], F32R, kind="ExternalInput").ap()
    wq = nc.dram_tensor("wq", [D, 8 * HD], F32R, kind="ExternalInput").ap()
    wk = nc.dram_tensor("wk", [D, 2 * HD], F32R, kind="ExternalInput").ap()
    wv = nc.dram_tensor("wv", [D, 2 * HD], F32R, kind="ExternalInput").ap()
    wo = nc.dram_tensor("wo", [8 * HD, D], F32R, kind="ExternalInput").ap()
    cosT2 = nc.dram_tensor("cosT2", [128, S], F32, kind="ExternalInput").ap()
    sinT2m = nc.dram_tensor("sinT2m", [128, S], F32, kind="ExternalInput").ap()
    tri = nc.dram_tensor("tri", [128, 128], F32, kind="ExternalInput").ap()
    ident = nc.dram_tensor("ident", [128, 64], F32R, kind="ExternalInput").ap()
    selA = nc.dram_tensor("selA", [128, 512], F32R, kind="ExternalInput").ap()
    selB = nc.dram_tensor("selB", [128, 512], F32R, kind="ExternalInput").ap()
    onescol = nc.dram_tensor("onescol", [128, 1], F32R, kind="ExternalInput").ap()
    zblk = nc.dram_tensor("zblk", [128, 128], F32R, kind="ExternalInput").ap()
    opart = nc.dram_tensor("opart", [S, D], F32, kind="ExternalOutput").ap()

    with tile.TileContext(nc) as tc:
        with (
            tc.tile_pool(name="persist", bufs=1) as pp,
            tc.tile_pool(name="consts", bufs=1) as cp,
        ):
            # persistent SBUF: q^T/k^T, attention outputs, small constants
            qT = [pp.tile([128, S], F32R, tag=f"qT{t}", name=f"qT{t}") for t in range(4)]
            kT = pp.tile([128, S], F32R, tag="kT")
            outT = [pp.tile([128, S], F32R, tag=f"outT{t}", name=f"outT{t}") for t in range(4)]
            denomA = pp.tile([128, S], F32, tag="denomA")
            denomB = pp.tile([128, S], F32, tag="denomB")
            trib = cp.tile([128, 128], F32, tag="trib")
            identb = cp.tile([128, 64], F32R, tag="identb")
            selAb = cp.tile([128, 512], F32R, tag="selAb")
            selBb = cp.tile([128, 512], F32R, tag="selBb")
            onesb = cp.tile([128, 1], F32R, tag="onesb")
            zblkb = cp.tile([128, 128], F32R, tag="zblkb")
            nc.sync.dma_start(trib[:], tri[:])
            nc.sync.dma_start(identb[:], ident[:])
            nc.sync.dma_start(selAb[:], selA[:])
            nc.sync.dma_start(selBb[:], selB[:])
            nc.sync.dma_start(onesb[:], onescol[:])
            nc.sync.dma_start(zblkb[:], zblk[:])
            nc.gpsimd.memset(denomA[:], 1.0)
            nc.gpsimd.memset(denomB[:], 1.0)

            vo = [[None] * NST, [None] * NST]
            with tc.tile_pool(name="vop", bufs=1) as vp:  # spans phases A..D
                with (
                    tc.tile_pool(name="ropec", bufs=1) as rcc,
                    tc.tile_pool(name="vtbuf", bufs=1) as vtb,
                ):
                    cosb = rcc.tile([128, S], F32, tag="cosb")
                    sinb = rcc.tile([128, S], F32, tag="sinb")
                    nc.sync.dma_start(cosb[:], cosT2[:])
                    nc.sync.dma_start(sinb[:], sinT2m[:])
                    vT = vtb.tile([128, S], F32R, tag="vT")

                    # ---------- Phase A: qkv^T = W^T @ x^T ----------
                    with (
                        tc.tile_pool(name="wts", bufs=1) as wp,
                        tc.tile_pool(name="xin", bufs=4) as xp,
                        tc.tile_pool(name="qkvps", bufs=6, space="PSUM") as pqkv,
                    ):
                        wqk = [wp.tile([128, 8 * HD], F32R, tag=f"wq{k}", name=f"wqk{k}") for k in range(NKT)]
                        wkk = [wp.tile([128, 2 * HD], F32R, tag=f"wk{k}", name=f"wkk{k}") for k in range(NKT)]
                        wvk = [wp.tile([128, 2 * HD], F32R, tag=f"wv{k}", name=f"wvk{k}") for k in range(NKT)]
                        for k in range(NKT):
                            r = slice(k * 128, (k + 1) * 128)
                            nc.sync.dma_start(wqk[k][:], wq[r, :])
                            nc.sync.dma_start(wkk[k][:], wk[r, :])
                            nc.sync.dma_start(wvk[k][:], wv[r, :])
                        for n in range(NCHUNK):
                            ncol = slice(n * 512, (n + 1) * 512)
                            accs = [pqkv.tile([128, 512], F32, tag="qkvacc", name=f"acc{n}_{m}") for m in range(6)]
                            for k in range(NKT):
                                xk = xp.tile([128, 512], F32R, tag="xk")
                                nc.sync.dma_start(xk[:], xT[k * 128:(k + 1) * 128, ncol])
                                st = k == 0
                                sp = k == NKT - 1
                                for t in range(4):
                                    nc.tensor.matmul(
                                        accs[t][:], wqk[k][:, t * 128:(t + 1) * 128],
                                        xk[:], start=st, stop=sp)
                                nc.tensor.matmul(accs[4][:], wkk[k][:], xk[:], start=st, stop=sp)
                                nc.tensor.matmul(accs[5][:], wvk[k][:], xk[:], start=st, stop=sp)
                            for t in range(4):
                                nc.vector.tensor_copy(qT[t][:, ncol], accs[t][:])
                            nc.vector.tensor_copy(kT[:, ncol], accs[4][:])
                            nc.vector.tensor_copy(vT[:, ncol], accs[5][:])

                    # ---------- Phase B: RoPE on q^T and k^T ----------
                    with tc.tile_pool(name="rope", bufs=2) as rp:
                        for tl in [*qT, kT]:
                            rot = rp.tile([128, S], F32, tag="rot")
                            tmp = rp.tile([128, S], F32, tag="tmp")
                            # rotate-half as partition-shifted copies (sign folded in sinb)
                            nc.gpsimd.tensor_copy(rot[0:32, :], tl[32:64, :])
                            nc.gpsimd.tensor_copy(rot[32:64, :], tl[0:32, :])
                            nc.gpsimd.tensor_copy(rot[64:96, :], tl[96:128, :])
                            nc.gpsimd.tensor_copy(rot[96:128, :], tl[64:96, :])
                            nc.vector.tensor_mul(tmp[:], tl[:], cosb[:])
                            nc.vector.tensor_mul(rot[:], rot[:], sinb[:])
                            nc.vector.tensor_add(tl[:], tmp[:], rot[:])

                    # ---------- Phase C: v natural tiles [128, 65] ----------
                    with tc.tile_pool(name="vtp", bufs=2, space="PSUM") as vtp:
                        for g in range(2):
                            for i in range(NST):
                                vps = vtp.tile([128, 64], F32R, tag="vps")
                                nc.tensor.transpose(
                                    vps[:], vT[g * 64:(g + 1) * 64, i * 128:(i + 1) * 128],
                                    identb[g * 64:(g + 1) * 64, :])
                                vt = vp.tile([128, 65], F32R, tag=f"vo{g}_{i}", name=f"vo{g}_{i}")
                                nc.vector.tensor_copy(vt[:, 0:64], vps[:])
                                nc.vector.tensor_copy(vt[:, 64:65], onesb[:])
                                vo[g][i] = vt

                # ---------- Phase D: attention ----------
                with (
                    tc.tile_pool(name="esb", bufs=10) as ep,
                    tc.tile_pool(name="sps", bufs=4, space="PSUM") as sp_,
                    tc.tile_pool(name="avp", bufs=3, space="PSUM") as ap_,
                ):
                    for t in range(4):
                        for j in range(NCHUNK):
                            jcol = slice(j * 512, (j + 1) * 512)
                            avs = []
                            for sub in range(2):
                                avs.append(ap_.tile([65, 512], F32, tag="avacc", name=f"av{t}_{j}_{sub}"))
                            for i in range(4 * j + 4):
                                c0 = max(0, 128 * (i - 4 * j))
                                ec0 = c0 if 512 - c0 >= 256 else 256
                                av0 = c0 if c0 < 384 else 256
                                for sub in range(2):
                                    pb = slice(64 * sub, 64 * sub + 64)
                                    g = sub
                                    ss = sp_.tile([128, 512], F32, tag="scps")
                                    nc.tensor.matmul(
                                        ss[:, ec0:512],
                                        kT[pb, i * 128:(i + 1) * 128],
                                        qT[t][pb, j * 512 + ec0:(j + 1) * 512],
                                        start=True, stop=True)
                                    es = ep.tile([128, 512], F32R, tag="es")
                                    nc.scalar.activation(
                                        es[:, c0:512], ss[:, c0:512], EXP, scale=SCALE)
                                    if i >= 4 * j:
                                        nc.vector.tensor_mul(
                                            es[:, c0:c0 + 128], es[:, c0:c0 + 128],
                                            trib[:])
                                    if c0 == 384:
                                        nc.vector.tensor_copy(es[:, 256:384], zblkb[:])
                                    nc.tensor.matmul(
                                        avs[sub][:, av0:512], vo[g][i][:],
                                        es[:, av0:512],
                                        start=(i == 0), stop=(i == 4 * j + 3))
                            for sub in range(2):
                                pb = slice(64 * sub, 64 * sub + 64)
                                nc.vector.tensor_copy(outT[t][pb, jcol], avs[sub][0:64, :])
                                dst = denomA if sub == 0 else denomB
                                nc.vector.tensor_copy(
                                    dst[32 * t:32 * t + 1, jcol], avs[sub][64:65, :])

            # ---------- Phase E: normalize + o_proj ----------
            with (
                tc.tile_pool(name="norm", bufs=2) as np_,
                tc.tile_pool(name="wop", bufs=1) as wop,
                tc.tile_pool(name="oout", bufs=3) as op,
                tc.tile_pool(name="bcps", bufs=2, space="PSUM") as bp_,
                tc.tile_pool(name="ops", bufs=4, space="PSUM") as opp,
                tc.tile_pool(name="rcp", bufs=1) as rcp,
            ):
                rcpf = rcp.tile([128, S], F32, tag="rcpf")
                rcprA = rcp.tile([128, S], F32R, tag="rcprA")
                rcprB = rcp.tile([128, S], F32R, tag="rcprB")
                for dt_, rr in ((denomA, rcprA), (denomB, rcprB)):
                    nc.vector.reciprocal(rcpf[:], dt_[:])
                    nc.vector.tensor_copy(rr[:], rcpf[:])
                for t in range(4):
                    tsl = slice(t * 128, (t + 1) * 128)
                    bcs = np_.tile([128, S], F32, tag="bcs")
                    for n in range(NCHUNK):
                        ncol = slice(n * 512, (n + 1) * 512)
                        bps = bp_.tile([128, 512], F32, tag="bps")
                        nc.tensor.matmul(
                            bps[:], selAb[:, tsl], rcprA[:, ncol],
                            start=True, stop=False)
                        nc.tensor.matmul(
                            bps[:], selBb[:, tsl], rcprB[:, ncol],
                            start=False, stop=True)
                        nc.vector.tensor_copy(bcs[:, ncol], bps[:])
                    nc.vector.tensor_mul(outT[t][:], outT[t][:], bcs[:])
                wot = [wop.tile([128, S], F32R, tag=f"wo{k}", name=f"wot{k}") for k in range(4)]
                for k in range(4):
                    nc.sync.dma_start(wot[k][:], wo[k * 128:(k + 1) * 128, :])
                for st in range(NST):
                    for dch in range(NCHUNK):
                        ops = opp.tile([128, 512], F32, tag="opps")
                        for kt in range(4):
                            nc.tensor.matmul(
                                ops[:], outT[kt][:, st * 128:(st + 1) * 128],
                                wot[kt][:, dch * 512:(dch + 1) * 512],
                                start=(kt == 0), stop=(kt == 3))
                        oo = op.tile([128, 512], F32, tag="oo")
                        nc.vector.tensor_copy(oo[:], ops[:])
                        nc.sync.dma_start(
                            opart[st * 128:(st + 1) * 128, dch * 512:(dch + 1) * 512],
                            oo[:])

    nc.compile()
    return nc


_PROGRAM = None


def _get_program():
    global _PROGRAM
    if _PROGRAM is None:
        _PROGRAM = _build_program()
    return _PROGRAM


def _make_in_maps(x, cos, sin, Wq, Wk, Wv, Wo):
    cosT = np.ascontiguousarray(cos.T.astype(np.float32))      # [64, S]
    sinT = np.ascontiguousarray(sin.T.astype(np.float32))
    cosT2 = np.tile(cosT, (2, 1))
    sinT2m = np.tile(np.concatenate([-sinT[:32], sinT[32:]], 0), (2, 1))
    tri = (np.arange(128)[None, :] >= np.arange(128)[:, None]).astype(np.float32)
    ident = np.tile(np.eye(64, dtype=np.float32), (2, 1))
    selA = np.zeros((128, 512), dtype=np.float32)
    selB = np.zeros((128, 512), dtype=np.float32)
    for t in range(4):
        selA[32 * t, 128 * t:128 * t + 64] = 1.0
        selB[32 * t, 128 * t + 64:128 * t + 128] = 1.0

    perm = [0, 4, 1, 5, 2, 6, 3, 7]
    in_maps = []
    for c in range(8):
        b, q = c // 4, c % 4
        idx = np.concatenate([np.arange(HD) + (8 * q + j) * HD for j in perm])
        in_maps.append({
            "xT": np.ascontiguousarray(x[b].T.astype(np.float32)),
            "wq": np.ascontiguousarray(Wq[:, idx].astype(np.float32)),
            "wk": np.ascontiguousarray(Wk[:, 2 * q * HD:(2 * q + 2) * HD].astype(np.float32)),
            "wv": np.ascontiguousarray(Wv[:, 2 * q * HD:(2 * q + 2) * HD].astype(np.float32)),
            "wo": np.ascontiguousarray(Wo[idx, :].astype(np.float32)),
            "cosT2": cosT2,
            "sinT2m": sinT2m,
            "tri": tri,
            "ident": ident,
            "selA": selA,
            "selB": selB,
            "onescol": np.ones((128, 1), dtype=np.float32),
            "zblk": np.zeros((128, 128), dtype=np.float32),
        })
    return in_maps


def _execute(in_maps, trace=False):
    nc = _get_program()
    return bass_utils.run_bass_kernel_spmd(
        nc, in_maps, core_ids=list(range(8)), trace=trace)


def kernel(x, cos, sin, Wq, Wk, Wv, Wo):
    in_maps = _make_in_maps(x, cos, sin, Wq, Wk, Wv, Wo)
    res = _execute(in_maps, trace=False)
    parts = [r["opart"] for r in res.results]
    out = np.empty((B, S, D), dtype=np.float32)
    for b in range(B):
        p = parts[4 * b:4 * b + 4]
        out[b] = (p[0] + p[1]) + (p[2] + p[3])
    return out



# revision 18
# speedup vs baseline: 2.1066x; 2.1066x over previous
"""GQA (RoPE + causal softmax) Trainium2 Bass kernel, 8-core SPMD.

Sharding: DP over batch (2) x TP over KV groups (4 quarters of heads).
Core c handles batch c//4 and head quarter c%4 (8 q-heads, 2 kv-heads).
Each core computes a partial o_proj ([S, D]); host sums 4 partials per batch.

Perf design (v2): all matmuls in bf16 (fp32 PSUM accumulate), and the
whole kernel is scheduled as one continuous TensorE stream so the PE HAM
clock gate stays at K=8/8 (2.4 GHz):
 - RoPE is fused per-512-chunk into the projection phase (DVE
   partition-shifted copies; no gpsimd, no separate phase).
 - v is transposed per-chunk on the PE right after its projection.
 - exp activations batch both KV heads of a k-block ([128, 2, 512-c0]).
 - o_proj + softmax-normalization for chunk j are interleaved into the
   attention stream of chunk j+1 as PE gap fillers.
"""

import os
import numpy as np
import ml_dtypes

import concourse.bass as bass
import concourse.mybir as mybir
import concourse.tile as tile
from concourse import bacc, bass_utils

B, S, D = 2, 2048, 2048
H, KV, HD = 32, 8, 64
REP = H // KV
SCALE = 1.0 / 8.0  # 1/sqrt(HD)

F32 = mybir.dt.float32
F32R = mybir.dt.float32r
BF16 = mybir.dt.bfloat16
EXP = mybir.ActivationFunctionType.Exp

NCH = S // 512           # 4 chunks of 512 positions
NKT = D // 128           # 16 k-tiles over D (contraction)
NST = S // 128           # 16 128-tiles over S

BF = ml_dtypes.bfloat16


def _build_program():
    nc = bacc.Bacc()

    xT = nc.dram_tensor("xT", [D, S], BF16, kind="ExternalInput").ap()
    wq = nc.dram_tensor("wq", [D, 8 * HD], BF16, kind="ExternalInput").ap()
    wk = nc.dram_tensor("wk", [D, 2 * HD], BF16, kind="ExternalInput").ap()
    wv = nc.dram_tensor("wv", [D, 2 * HD], BF16, kind="ExternalInput").ap()
    wo = nc.dram_tensor("wo", [8 * HD, D], BF16, kind="ExternalInput").ap()
    cosT2 = nc.dram_tensor("cosT2", [128, S], F32, kind="ExternalInput").ap()
    sinT2m = nc.dram_tensor("sinT2m", [128, S], F32, kind="ExternalInput").ap()
    tri2 = nc.dram_tensor("tri2", [128, 2, 128], BF16, kind="ExternalInput").ap()
    ident = nc.dram_tensor("ident", [128, 64], F32R, kind="ExternalInput").ap()
    selA = nc.dram_tensor("selA", [128, 512], F32R, kind="ExternalInput").ap()
    selB = nc.dram_tensor("selB", [128, 512], F32R, kind="ExternalInput").ap()
    opart = nc.dram_tensor("opart", [S, D], F32, kind="ExternalOutput").ap()

    with tile.TileContext(nc) as tc:
        with (
            tc.tile_pool(name="persist", bufs=1) as pp,
            tc.tile_pool(name="consts", bufs=1) as cp,
            tc.tile_pool(name="wts", bufs=1) as wp,
        ):
            # ---- persistent SBUF ----
            qT = [pp.tile([128, S], BF16, tag=f"qT{t}", name=f"qT{t}") for t in range(4)]
            kT = pp.tile([128, S], BF16, tag="kT")
            outT = [pp.tile([128, S], BF16, tag=f"oT{t}", name=f"outT{t}") for t in range(4)]
            # voAll[:, 65*(2i+g) : 65*(2i+g)+65] = v^T tile for k-block i,
            # kv-head g, with a ones column at col 64 (denominator trick).
            voAll = pp.tile([128, NST * 2 * 65], BF16, tag="voAll")
            # denominator rows: denomA/B[32*t] = sum_k attn for (t, sub)
            denomA = pp.tile([128, S], F32, tag="denomA")
            denomB = pp.tile([128, S], F32, tag="denomB")
            cosb = cp.tile([128, S], F32, tag="cosb")
            sinb = cp.tile([128, S], F32, tag="sinb")
            trib = cp.tile([128, 2, 128], BF16, tag="trib")
            identb = cp.tile([128, 64], F32R, tag="identb")
            selAb = cp.tile([128, 512], F32R, tag="selAb")
            selBb = cp.tile([128, 512], F32R, tag="selBb")

            # weights (bf16): wq tiles [128, 512], wk/wv tiles [128, 128]
            wqk = [wp.tile([128, 8 * HD], BF16, tag=f"wq{k}", name=f"wqk{k}") for k in range(NKT)]
            wkk = [wp.tile([128, 2 * HD], BF16, tag=f"wk{k}", name=f"wkk{k}") for k in range(NKT)]
            wvk = [wp.tile([128, 2 * HD], BF16, tag=f"wv{k}", name=f"wvk{k}") for k in range(NKT)]
            wot = [wp.tile([128, S], BF16, tag=f"wo{k}", name=f"wot{k}") for k in range(4)]

            # ones columns of voAll
            ones_view = voAll[:].rearrange("p (n c) -> p n c", c=65)[:, :, 64:65]
            nc.vector.memset(ones_view, 1.0)
            nc.vector.memset(denomA[:], 1.0)
            nc.vector.memset(denomB[:], 1.0)

            vTc = [None] * NCH  # per-chunk v^T f32r [128, 512]

            # =========== Phase A: projections + fused RoPE + v^T ===========
            with (
                tc.tile_pool(name="xin", bufs=1) as xp,
                tc.tile_pool(name="rope", bufs=4) as rp,
                tc.tile_pool(name="vtc", bufs=2) as vcp,
                tc.tile_pool(name="accs", bufs=1, space="PSUM") as pacc,
                tc.tile_pool(name="vtp", bufs=2, space="PSUM") as pvt,
            ):
                # resident x^T (bf16, 16 x [128, S]); DMA interleaved with
                # weight tiles, k-ascending, so the first matmul starts ~us in.
                xtb = [xp.tile([128, S], BF16, tag=f"xt{k}", name=f"xtb{k}")
                       for k in range(NKT)]
                for k in range(NKT):
                    r = slice(k * 128, (k + 1) * 128)
                    nc.sync.dma_start(xtb[k][:], xT[r, :])
                    nc.sync.dma_start(wqk[k][:], wq[r, :])
                    nc.sync.dma_start(wkk[k][:], wk[r, :])
                    nc.sync.dma_start(wvk[k][:], wv[r, :])
                    if k == 1:
                        nc.sync.dma_start(cosb[:], cosT2[:])
                        nc.sync.dma_start(sinb[:], sinT2m[:])
                    if k == 3:
                        nc.sync.dma_start(identb[:], ident[:])
                def emit_transposes(n):
                    # v^T natural tiles for chunk n: 8 PE transposes (each
                    # into its own PSUM bank; slice-sharing a bank hangs HW)
                    for ii in range(4):
                        i = 4 * n + ii
                        for g in range(2):
                            vtp = pvt.tile([128, 64], F32R, tag="vtp",
                                           name=f"vtp{n}_{ii}_{g}")
                            nc.tensor.transpose(
                                vtp[:],
                                vTc[n][g * 64:(g + 1) * 64, ii * 128:(ii + 1) * 128],
                                identb[g * 64:(g + 1) * 64, :])
                            off = 65 * (2 * i + g)
                            nc.scalar.copy(voAll[:, off:off + 64], vtp[:])

                for n in range(NCH):
                    ncol = slice(n * 512, (n + 1) * 512)
                    accs = [pacc.tile([128, 512], F32, tag=f"acc{m}", name=f"acc{n}_{m}")
                            for m in range(6)]
                    # per-output dk loops: acc slot m is reused only after its
                    # (short) DVE RoPE chain, so the PE never waits at chunk
                    # boundaries.
                    for m in range(6):
                        for k in range(NKT):
                            if m < 4:
                                wsel = wqk[k][:, m * 128:(m + 1) * 128]
                            elif m == 4:
                                wsel = wkk[k][:]
                            else:
                                wsel = wvk[k][:]
                            nc.tensor.matmul(
                                accs[m][:], wsel, xtb[k][:, ncol],
                                start=(k == 0), stop=(k == NKT - 1))
                        if m == 0 and n > 0 and not os.environ.get("K2_NOVT"):
                            emit_transposes(n - 1)
                        if m < 5:
                            # fused RoPE: q tile m (or k tile), frees acc m
                            acc = accs[m]
                            dstT = qT[m] if m < 4 else kT
                            if os.environ.get("K2_NOROPE"):
                                nc.vector.tensor_copy(dstT[:, ncol], acc[:])
                                continue
                            rot = rp.tile([128, 512], F32, tag="rot")
                            tmp = rp.tile([128, 512], F32, tag="tmp")
                            for a, b in ((0, 32), (32, 0), (64, 96), (96, 64)):
                                nc.vector.tensor_copy(rot[a:a + 32, :], acc[b:b + 32, :])
                            nc.vector.tensor_mul(tmp[:], acc[:], cosb[:, ncol])
                            nc.vector.tensor_mul(rot[:], rot[:], sinb[:, ncol])
                            nc.vector.tensor_add(dstT[:, ncol], tmp[:], rot[:])
                        else:
                            # v chunk: PSUM -> f32r SBUF on the idle scalar eng
                            vTc[n] = vcp.tile([128, 512], F32R, tag="vTc",
                                              name=f"vTc{n}")
                            nc.scalar.copy(vTc[n][:], accs[5][:])
                if not os.environ.get("K2_NOVT"):
                    emit_transposes(NCH - 1)

            if os.environ.get("K2_PHASE") == "A":
                with tc.tile_pool(name="dbg", bufs=2) as dbp:
                    for t in range(4):
                        dt_ = dbp.tile([128, S], F32, tag=f"dbg", name=f"dbg{t}")
                        nc.vector.tensor_copy(dt_[:], qT[t][:])
                        nc.sync.dma_start(opart[t * 128:(t + 1) * 128, :], dt_[:])
            SKIP_D = os.environ.get("K2_PHASE") == "A"

            # =========== Phase D/E: attention + normalize + o_proj ===========
            with (
                tc.tile_pool(name="esb", bufs=5) as ep,
                tc.tile_pool(name="oob", bufs=4) as op,
                tc.tile_pool(name="bcb", bufs=2) as bp,
                tc.tile_pool(name="rcpp", bufs=2) as rpp,
                tc.tile_pool(name="scps", bufs=2, space="PSUM") as psc,
                tc.tile_pool(name="pd", bufs=4, space="PSUM") as pd,
            ):
                fillers = []  # deferred (bcs+normalize+o_proj) closures
                NJ = 0 if SKIP_D else NCH

                def drain(kmax):
                    for _ in range(min(kmax, len(fillers))):
                        fillers.pop(0)()

                def make_bcs(t, j, rcpA, rcpB):
                    jcol = slice(j * 512, (j + 1) * 512)

                    def go():
                        bps = pd.tile([128, 512], F32, tag="pd", name=f"bps{t}_{j}")
                        tsl = slice(t * 128, (t + 1) * 128)
                        nc.tensor.matmul(
                            bps[:], selAb[:, tsl], rcpA[:], start=True, stop=False)
                        nc.tensor.matmul(
                            bps[:], selBb[:, tsl], rcpB[:], start=False, stop=True)
                        bcsb = bp.tile([128, 512], BF16, tag="bcsb")
                        nc.vector.tensor_copy(bcsb[:], bps[:])
                        nc.vector.tensor_mul(outT[t][:, jcol], outT[t][:, jcol], bcsb[:])
                    return go

                def make_oproj(st, dch):
                    def go():
                        ops = pd.tile([128, 512], F32, tag="pd", name=f"ops{st}_{dch}")
                        for kt in range(4):
                            nc.tensor.matmul(
                                ops[:], outT[kt][:, st * 128:(st + 1) * 128],
                                wot[kt][:, dch * 512:(dch + 1) * 512],
                                start=(kt == 0), stop=(kt == 3))
                        oo = op.tile([128, 512], F32, tag="oo")
                        nc.vector.tensor_copy(oo[:], ops[:])
                        nc.sync.dma_start(
                            opart[st * 128:(st + 1) * 128, dch * 512:(dch + 1) * 512],
                            oo[:])
                    return go

                if not SKIP_D:
                    for k in range(4):
                        nc.sync.dma_start(wot[k][:], wo[k * 128:(k + 1) * 128, :])
                    nc.sync.dma_start(trib[:], tri2[:])
                    nc.sync.dma_start(selAb[:], selA[:])
                    nc.sync.dma_start(selBb[:], selB[:])
                for j in ([] if SKIP_D else [3, 2, 1, 0]):
                    jcol = slice(j * 512, (j + 1) * 512)
                    ni = 4 * j + 4
                    for t in range(4):
                        avs = [pd.tile([65, 512], F32, tag="pd", name=f"av{t}_{j}_{s_}")
                               for s_ in range(2)]
                        pend = {}

                        def emit_S(i, t=t, j=j, pend=pend):
                            c0 = max(0, 128 * (i - 4 * j))
                            sps = psc.tile([128, 2, 512], F32, tag="sc")
                            for sub in range(2):
                                pb = slice(64 * sub, 64 * sub + 64)
                                nc.tensor.matmul(
                                    sps[:, sub, c0:512],
                                    kT[pb, i * 128:(i + 1) * 128],
                                    qT[t][pb, j * 512 + c0:(j + 1) * 512],
                                    start=True, stop=True)
                            es = ep.tile([128, 2, 512], BF16, tag="es")
                            nc.scalar.activation(
                                es[:, :, c0:512], sps[:, :, c0:512], EXP, scale=SCALE)
                            if i >= 4 * j:
                                nc.vector.tensor_mul(
                                    es[:, :, c0:c0 + 128], es[:, :, c0:c0 + 128],
                                    trib[:])
                            pend[i] = (es, c0)

                        def emit_A(i, t=t, j=j, ni=ni, avs=avs, pend=pend):
                            es, c0 = pend.pop(i)
                            for sub in range(2):
                                nc.tensor.matmul(
                                    avs[sub][:, c0:512],
                                    voAll[:, 65 * (2 * i + sub):65 * (2 * i + sub) + 65],
                                    es[:, sub, c0:512],
                                    start=(i == 0), stop=(i == ni - 1))

                        emit_S(0)
                        if ni > 1:
                            emit_S(1)
                        for i in range(ni):
                            if not os.environ.get("K2_NOFILL"):
                                drain(2)
                            if i + 2 < ni:
                                emit_S(i + 2)
                            emit_A(i)
                        # attention outputs + denominators for (t, j)
                        for sub in range(2):
                            pb = slice(64 * sub, 64 * sub + 64)
                            nc.vector.tensor_copy(outT[t][pb, jcol], avs[sub][0:64, :])
                            dn = denomA if sub == 0 else denomB
                            nc.vector.tensor_copy(
                                dn[32 * t:32 * t + 1, jcol], avs[sub][64:65, :])

                    # reciprocal of the denominator rows for this chunk
                    rcpA = rpp.tile([128, 512], F32R, tag="rcpA", name=f"rcpA{j}")
                    rcpB = rpp.tile([128, 512], F32R, tag="rcpB", name=f"rcpB{j}")
                    for rcp_, dn_ in ((rcpA, denomA), (rcpB, denomB)):
                        rf = rpp.tile([128, 512], F32, tag="rcpf")
                        nc.vector.reciprocal_approx_fast(rf[:], dn_[:, jcol])
                        nc.vector.tensor_copy(rcp_[:], rf[:])
                    for t in range(4):
                        fillers.append(make_bcs(t, j, rcpA, rcpB))
                    for st in range(4 * j, 4 * j + 4):
                        for dch in range(NCH):
                            fillers.append(make_oproj(st, dch))
                    if os.environ.get("K2_NOFILL"):
                        drain(len(fillers))
                drain(len(fillers))

    nc.compile()
    return nc


_PROGRAM = None


def _get_program():
    global _PROGRAM
    if _PROGRAM is None:
        _PROGRAM = _build_program()
    return _PROGRAM


def _make_in_maps(x, cos, sin, Wq, Wk, Wv, Wo):
    cosT = np.ascontiguousarray(cos.T.astype(np.float32))      # [64, S]
    sinT = np.ascontiguousarray(sin.T.astype(np.float32))
    cosT2 = np.tile(cosT, (2, 1))
    sinT2m = np.tile(np.concatenate([-sinT[:32], sinT[32:]], 0), (2, 1))
    tri = (np.arange(128)[None, :] >= np.arange(128)[:, None]).astype(BF)
    tri2 = np.ascontiguousarray(
        np.broadcast_to(tri[:, None, :], (128, 2, 128)))
    ident = np.tile(np.eye(64, dtype=np.float32), (2, 1))
    # selA/selB broadcast reciprocal row 32t onto partitions of outT[t]:
    # partitions 0-63 <- denomA (sub 0), 64-127 <- denomB (sub 1).
    selA = np.zeros((128, 512), dtype=np.float32)
    selB = np.zeros((128, 512), dtype=np.float32)
    for t in range(4):
        selA[32 * t, 128 * t:128 * t + 64] = 1.0
        selB[32 * t, 128 * t + 64:128 * t + 128] = 1.0

    perm = [0, 4, 1, 5, 2, 6, 3, 7]
    in_maps = []
    for c in range(8):
        b, q = c // 4, c % 4
        idx = np.concatenate([np.arange(HD) + (8 * q + j) * HD for j in perm])
        in_maps.append({
            "xT": np.ascontiguousarray(x[b].T).astype(BF),
            "wq": np.ascontiguousarray(Wq[:, idx]).astype(BF),
            "wk": np.ascontiguousarray(Wk[:, 2 * q * HD:(2 * q + 2) * HD]).astype(BF),
            "wv": np.ascontiguousarray(Wv[:, 2 * q * HD:(2 * q + 2) * HD]).astype(BF),
            "wo": np.ascontiguousarray(Wo[idx, :]).astype(BF),
            "cosT2": cosT2,
            "sinT2m": sinT2m,
            "tri2": tri2,
            "ident": ident,
            "selA": selA,
            "selB": selB,
        })
    return in_maps


def _execute(in_maps, trace=False):
    nc = _get_program()
    return bass_utils.run_bass_kernel_spmd(
        nc, in_maps, core_ids=list(range(8)), trace=trace)


def kernel(x, cos, sin, Wq, Wk, Wv, Wo):
    in_maps = _make_in_maps(x, cos, sin, Wq, Wk, Wv, Wo)
    res = _execute(in_maps, trace=False)
    parts = [r["opart"] for r in res.results]
    out = np.empty((B, S, D), dtype=np.float32)
    for b in range(B):
        p = parts[4 * b:4 * b + 4]
        out[b] = (p[0] + p[1]) + (p[2] + p[3])
    return out
